# revision 1
# baseline (speedup 1.0000x reference)
"""Trainium2 Bass kernel for nn_LSH: ret[o] = sum_{s,a} x[s] * w[o,s,a].

x: [1, 4096] f32, weights: [512, 4096, 128] f32 -> ret: [512] f32.

Sharding: out_dim 512 is split 64-per-core across 8 cores; x is replicated.
Per core the 64x4096x128 f32 slice (128 MiB) is streamed from HBM as a flat
[128, 262144] layout (partition p = o=p//2, s in [(p%2)*2048, ...+2048)).
Compute per chunk: DVE segmented reduce over the innermost a=128, giving
T[p, s_local]; then one elementwise multiply with the replicated x pattern,
a free-dim reduce, and a tiny pairing matmul to fold partition pairs
(2o, 2o+1) into ret[o].
"""

import sys

sys.path.insert(0, "/opt/trn_rl_repo")

import numpy as np

import concourse.bass as bass
import concourse.mybir as mybir
import concourse.tile as tile
from concourse import bacc
from concourse.bass_utils import run_bass_kernel_spmd

P = 128
O_PER_CORE = 64
N_CORES = 8
S = 4096
A = 128
COLS = O_PER_CORE * S * A // P  # 262144 per-partition row length
SLOC = 2048  # s-values covered by each partition
CHUNK = 8192  # columns per DMA/reduce chunk
NCHUNK = COLS // CHUNK  # 32
NSEG = CHUNK // A  # 64 s-values produced per chunk

_CACHED_NC = None


def _build_nc():
    nc = bacc.Bacc(
        "TRN2",
        target_bir_lowering=False,
        debug=False,
        num_devices=N_CORES,
    )
    w = nc.dram_tensor("w", [P, COLS], mybir.dt.float32, kind="ExternalInput").ap()
    xt = nc.dram_tensor("xt", [P, SLOC], mybir.dt.float32, kind="ExternalInput").ap()
    pmat = nc.dram_tensor(
        "pmat", [P, O_PER_CORE], mybir.dt.float32, kind="ExternalInput"
    ).ap()
    out = nc.dram_tensor(
        "out", [O_PER_CORE, 1], mybir.dt.float32, kind="ExternalOutput"
    ).ap()

    with tile.TileContext(nc) as tc:
        with (
            tc.tile_pool(name="wp", bufs=3) as wp,
            tc.tile_pool(name="const", bufs=1) as constp,
            tc.tile_pool(name="accp", bufs=1) as accp,
            tc.tile_pool(name="psum", bufs=1, space="PSUM") as psp,
        ):
            xt_t = constp.tile([P, SLOC], mybir.dt.float32)
            nc.sync.dma_start(xt_t[:], xt[:])
            pm_t = constp.tile([P, O_PER_CORE], mybir.dt.float32)
            nc.sync.dma_start(pm_t[:], pmat[:])

            acc = accp.tile([P, SLOC], mybir.dt.float32)
            for k in range(NCHUNK):
                wt = wp.tile([P, CHUNK], mybir.dt.float32)
                nc.sync.dma_start(wt[:], w[:, k * CHUNK : (k + 1) * CHUNK])
                seg = wt[:].rearrange("p (n a) -> p n a", a=A)
                nc.vector.tensor_reduce(
                    acc[:, k * NSEG : (k + 1) * NSEG],
                    seg,
                    axis=mybir.AxisListType.X,
                    op=mybir.AluOpType.add,
                )

            accx = accp.tile([P, SLOC], mybir.dt.float32)
            nc.vector.tensor_mul(accx[:], acc[:], xt_t[:])
            v = accp.tile([P, 1], mybir.dt.float32)
            nc.vector.tensor_reduce(
                v[:], accx[:], axis=mybir.AxisListType.X, op=mybir.AluOpType.add
            )
            ps = psp.tile([O_PER_CORE, 1], mybir.dt.float32)
            nc.tensor.matmul(ps[:], pm_t[:], v[:], start=True, stop=True)
            res = accp.tile([O_PER_CORE, 1], mybir.dt.float32)
            nc.scalar.copy(res[:], ps[:])
            nc.sync.dma_start(out[:], res[:])

    nc.compile()
    return nc


def _get_nc():
    global _CACHED_NC
    if _CACHED_NC is None:
        _CACHED_NC = _build_nc()
    return _CACHED_NC


def _in_maps(x, weights):
    x = np.ascontiguousarray(np.asarray(x, dtype=np.float32))
    weights = np.asarray(weights, dtype=np.float32)
    xt = np.tile(x.reshape(2, SLOC), (P // 2, 1))
    pmat = np.zeros((P, O_PER_CORE), dtype=np.float32)
    pmat[np.arange(P), np.arange(P) // 2] = 1.0
    maps = []
    for c in range(N_CORES):
        wc = np.ascontiguousarray(
            weights[c * O_PER_CORE : (c + 1) * O_PER_CORE]
        ).reshape(P, COLS)
        maps.append({"w": wc, "xt": xt, "pmat": pmat})
    return maps


def run(x, weights, trace=False):
    """Run on hardware; returns (ret[512], BassKernelResults)."""
    nc = _get_nc()
    res = run_bass_kernel_spmd(
        nc, _in_maps(x, weights), list(range(N_CORES)), trace=trace
    )
    ret = np.concatenate(
        [res.results[c]["out"].reshape(O_PER_CORE) for c in range(N_CORES)]
    ).astype(np.float32)
    return ret, res


def kernel(x, weights):
    ret, _ = run(x, weights)
    return ret
